# revision 10
# baseline (speedup 1.0000x reference)
"""Context-aware attention kernel for 8 Trainium2 NeuronCores.

Reference computation (B=128, LQ=32, LK=128, D=1024, H=16, DK=64):
  K_cat = concat(K_seq, Q_seq) on the sequence axis  -> [B, 160, D]
  Qh/Kh/Vh = per-head projections, custom exp-based masked attention
  out[b,q,:] = (sum_k mexp[q,k] Vh[k,:]) / (sum_k mexp[q,k] + 1e-8)
  with mexp = exp(QK^T/8) * mask.

Sharding: data-parallel over batch, 16 batches per core. Inside a core the
work is split into 2 halves of 8 batches; each half is processed as
projection GEMMs (bf16 inputs, fp32 PSUM accumulate) followed by attention.
Scores are computed transposed ([k, q] layout) so that the context matmul
takes the masked-exp tile directly as the stationary operand — no on-chip
transposes anywhere (X^T and W^T are prepared on the host). V carries an
extra all-ones column per head so the normalization denominator falls out
of the context matmul for free.
"""

import os
import sys

for _p in ("/opt/trn_rl_repo", "/root/.axon_site/_ro/trn_rl_repo"):
    if os.path.isdir(_p) and _p not in sys.path:
        sys.path.insert(0, _p)

import numpy as np
import ml_dtypes

import concourse.bacc as bacc
import concourse.mybir as mybir
import concourse.tile as tile
from concourse.bass_utils import run_bass_kernel_spmd

B, LQ, LK, D, H, DK = 128, 32, 128, 1024, 16, 64
L = LK + LQ              # 160 keys per batch after concat
NCORES = 8
NB = B // NCORES         # 16 batches per core
NHALF = 2
BH = NB // NHALF         # 8 batches per half
QUAD = 4                 # batches packed into one 128-partition group
NQ_H = BH // QUAD        # quads per half
HC = DK + 1              # per-head V columns incl. the ones column
SCALE = 1.0 / np.sqrt(float(DK))

BF = mybir.dt.bfloat16
F32 = mybir.dt.float32
EXP = mybir.ActivationFunctionType.Exp

_NC = None
_LAST_RESULT = None


def _build():
    nc = bacc.Bacc(
        "TRN2",
        target_bir_lowering=False,
        debug=False,
        enable_asserts=False,
        num_devices=NCORES,
    )
    xt = nc.dram_tensor("xt", [D, NB * L], BF, kind="ExternalInput").ap()
    xvt = nc.dram_tensor("xvt", [D, NB * LK], BF, kind="ExternalInput").ap()
    xqt = nc.dram_tensor("xqt", [D, NB * LQ], BF, kind="ExternalInput").ap()
    wqt = nc.dram_tensor("wqt", [D, D], BF, kind="ExternalInput").ap()
    wkt = nc.dram_tensor("wkt", [D, D], BF, kind="ExternalInput").ap()
    wvt = nc.dram_tensor("wvt", [D, D], BF, kind="ExternalInput").ap()
    bqd = nc.dram_tensor("bq", [8, 128, 1], F32, kind="ExternalInput").ap()
    bkd = nc.dram_tensor("bk", [8, 128, 1], F32, kind="ExternalInput").ap()
    bvd = nc.dram_tensor("bvrow", [1, D], BF, kind="ExternalInput").ap()
    mad = nc.dram_tensor("maska", [NB, LK, LQ], BF, kind="ExternalInput").ap()
    mbd = nc.dram_tensor("maskb", [NB // QUAD, QUAD * LQ, LQ], BF, kind="ExternalInput").ap()
    outd = nc.dram_tensor("out", [NB * LQ, D], F32, kind="ExternalOutput").ap()

    from contextlib import ExitStack

    with tile.TileContext(nc) as tc, ExitStack() as st:
            pers = st.enter_context(tc.tile_pool(name="pers", bufs=1))
            xtp = st.enter_context(tc.tile_pool(name="xtp", bufs=16))
            khtp = st.enter_context(tc.tile_pool(name="khtp", bufs=2))
            qhtp = st.enter_context(tc.tile_pool(name="qhtp", bufs=2))
            vhap = st.enter_context(tc.tile_pool(name="vhap", bufs=12))
            vhbp = st.enter_context(tc.tile_pool(name="vhbp", bufs=3))
            mapl = st.enter_context(tc.tile_pool(name="mapl", bufs=12))
            mbpl = st.enter_context(tc.tile_pool(name="mbpl", bufs=3))
            meap = st.enter_context(tc.tile_pool(name="meap", bufs=8))
            mebp = st.enter_context(tc.tile_pool(name="mebp", bufs=2))
            outp = st.enter_context(tc.tile_pool(name="outp", bufs=3))
            mscp = st.enter_context(tc.tile_pool(name="mscp", bufs=4))
            pproj = st.enter_context(tc.tile_pool(name="pproj", bufs=2, space="PSUM"))
            psa = st.enter_context(tc.tile_pool(name="psa", bufs=2, space="PSUM"))
            psb = st.enter_context(tc.tile_pool(name="psb", bufs=2, space="PSUM"))
            pctx = st.enter_context(tc.tile_pool(name="pctx", bufs=2, space="PSUM"))
            # ---- persistent weights / biases -------------------------------
            wq_sb, wk_sb, wv_sb = [], [], []
            for d in range(8):
                for lst, src, nm in ((wq_sb, wqt, "wq"), (wk_sb, wkt, "wk"), (wv_sb, wvt, "wv")):
                    t = pers.tile([128, D], BF, tag=f"{nm}{d}")
                    nc.sync.dma_start(t[:], src[d * 128 : (d + 1) * 128, :])
                    lst.append(t)
            bq_sb, bk_sb = [], []
            for o in range(8):
                t = pers.tile([128, 1], F32, tag=f"bq{o}")
                nc.sync.dma_start(t[:], bqd[o])
                bq_sb.append(t)
                t = pers.tile([128, 1], F32, tag=f"bk{o}")
                nc.sync.dma_start(t[:], bkd[o])
                bk_sb.append(t)
            # broadcast bv across partitions via a K=1 matmul with ones
            ones1 = pers.tile([1, 128], BF, tag="ones1")
            nc.vector.memset(ones1[:], 1.0)
            bvr = pers.tile([1, D], BF, tag="bvr")
            nc.sync.dma_start(bvr[:], bvd[:])
            bvb = pers.tile([128, D], F32, tag="bvb")
            for oc in range(2):
                ps = pproj.tile([128, 512], F32, tag="proj")
                nc.tensor.matmul(ps[:], ones1[:], bvr[:, oc * 512 : (oc + 1) * 512],
                                 start=True, stop=True)
                nc.vector.tensor_copy(bvb[:, oc * 512 : (oc + 1) * 512], ps[:])
            bvb_v = bvb.rearrange("p (h c) -> p h c", c=DK)

            for half in range(NHALF):
                b0 = half * BH  # core-local first batch of this half
                # ---- masks -------------------------------------------------
                ma_sb = {}
                for j in range(BH):
                    t = mapl.tile([LK, LQ], BF, tag="ma")
                    nc.sync.dma_start(t[:], mad[b0 + j])
                    ma_sb[b0 + j] = t
                mb_sb = {}
                for qd in range(NQ_H):
                    gq = half * NQ_H + qd
                    t = mbpl.tile([QUAD * LQ, LQ], BF, tag="mb")
                    nc.sync.dma_start(t[:], mbd[gq])
                    mb_sb[gq] = t

                kht_sb = [khtp.tile([128, BH * L], BF, name=f"kht{o}", tag=f"kht{o}") for o in range(8)]
                qht_sb = [qhtp.tile([128, BH * LQ], BF, name=f"qht{o}", tag=f"qht{o}") for o in range(8)]
                vha_sb, vhb_sb = {}, {}

                # ---- projections (per quad of 4 batches) -------------------
                for qd in range(NQ_H):
                    gq = half * NQ_H + qd
                    c0 = gq * QUAD * L
                    xt_sb = [xtp.tile([128, QUAD * L], BF, name="xt", tag="xt") for _ in range(8)]
                    xv_sb = [xtp.tile([128, QUAD * LK], BF, name="xv", tag="xv") for _ in range(8)]
                    xq_sb = [xtp.tile([128, QUAD * LQ], BF, name="xq", tag="xq") for _ in range(8)]
                    cv0 = gq * QUAD * LK
                    cq0 = gq * QUAD * LQ
                    for d in range(8):
                        nc.sync.dma_start(xt_sb[d][:], xt[d * 128 : (d + 1) * 128, c0 : c0 + QUAD * L])
                        nc.sync.dma_start(xv_sb[d][:], xvt[d * 128 : (d + 1) * 128, cv0 : cv0 + QUAD * LK])
                        nc.sync.dma_start(xq_sb[d][:], xqt[d * 128 : (d + 1) * 128, cq0 : cq0 + QUAD * LQ])

                    # K^T projection: [o, t] = Wk @ X^T, 2 chunks of 320 cols
                    for o in range(8):
                        for s in range(2):
                            ps = pproj.tile([128, 512], F32, tag="proj")
                            for d in range(8):
                                nc.tensor.matmul(
                                    ps[:, 0:320],
                                    wk_sb[d][:, o * 128 : (o + 1) * 128],
                                    xt_sb[d][:, s * 320 : (s + 1) * 320],
                                    start=(d == 0), stop=(d == 7),
                                )
                            nc.vector.tensor_scalar_add(
                                kht_sb[o][:, qd * QUAD * L + s * 320 : qd * QUAD * L + (s + 1) * 320],
                                ps[:, 0:320], bk_sb[o][:],
                            )
                    # Q^T projection (only the 32 query-token cols per batch)
                    for o in range(8):
                        ps = pproj.tile([128, 512], F32, tag="proj")
                        for d in range(8):
                            nc.tensor.matmul(
                                ps[:, 0 : QUAD * LQ],
                                wq_sb[d][:, o * 128 : (o + 1) * 128],
                                xq_sb[d][:], start=(d == 0), stop=(d == 7),
                            )
                        nc.vector.tensor_scalar_add(
                            qht_sb[o][:, qd * QUAD * LQ : (qd + 1) * QUAD * LQ],
                            ps[:, 0 : QUAD * LQ], bq_sb[o][:],
                        )
                    # V projection, natural [t, o] layout; per-head ones column
                    for j in range(QUAD):
                        gb = gq * QUAD + j
                        va = vhap.tile([128, H * HC], BF, tag="vha")
                        vav = va.rearrange("p (h c) -> p h c", c=HC)
                        nc.vector.memset(vav[:, :, DK : DK + 1], 1.0)
                        for oc in range(2):
                            ps = pproj.tile([128, 512], F32, tag="proj")
                            for d in range(8):
                                nc.tensor.matmul(
                                    ps[:],
                                    xv_sb[d][:, j * LK : (j + 1) * LK],
                                    wv_sb[d][:, oc * 512 : (oc + 1) * 512],
                                    start=(d == 0), stop=(d == 7),
                                )
                            nc.vector.tensor_add(
                                vav[:, oc * 8 : (oc + 1) * 8, 0:DK],
                                ps[:].rearrange("p (h c) -> p h c", c=DK),
                                bvb_v[:, oc * 8 : (oc + 1) * 8, :],
                            )
                        vha_sb[gb] = va
                    # V projection for the 32 concat-query rows of the 4
                    # batches, gathered so batch j lands on partitions 32j.
                    vb = vhbp.tile([128, H * HC], BF, tag="vhb")
                    vbv = vb.rearrange("p (h c) -> p h c", c=HC)
                    nc.vector.memset(vbv[:, :, DK : DK + 1], 1.0)
                    for oc in range(2):
                        ps = pproj.tile([128, 512], F32, tag="proj")
                        for d in range(8):
                            nc.tensor.matmul(
                                ps[:], xq_sb[d][:],
                                wv_sb[d][:, oc * 512 : (oc + 1) * 512],
                                start=(d == 0), stop=(d == 7),
                            )
                        nc.vector.tensor_add(
                            vbv[:, oc * 8 : (oc + 1) * 8, 0:DK],
                            ps[:].rearrange("p (h c) -> p h c", c=DK),
                            bvb_v[:, oc * 8 : (oc + 1) * 8, :],
                        )
                    vhb_sb[gq] = vb

                # ---- attention ---------------------------------------------
                for qd in range(NQ_H):
                    gq = half * NQ_H + qd
                    outq = outp.tile([128, D], F32, tag="outq")
                    for h in range(H):
                        ot, h2 = h // 2, 64 * (h % 2)
                        sb4 = psb.tile([QUAD * LQ, LQ], F32, tag="sb")
                        mexa = []
                        for j in range(QUAD):
                            bl = qd * QUAD + j  # half-local batch index
                            sa = psa.tile([LK, LQ], F32, tag="sa")
                            # scores^T[k, q] for the 128 body keys
                            nc.tensor.matmul(
                                sa[:],
                                kht_sb[ot][h2 : h2 + 64, bl * L : bl * L + LK],
                                qht_sb[ot][h2 : h2 + 64, bl * LQ : (bl + 1) * LQ],
                                start=True, stop=True, tile_position=(h2, 0),
                            )
                            # scores^T for the 32 concat-query keys, batch j
                            # on psum partitions 32j via PE column groups
                            nc.tensor.matmul(
                                sb4[32 * j : 32 * (j + 1), :],
                                kht_sb[ot][h2 : h2 + 64, bl * L + LK : (bl + 1) * L],
                                qht_sb[ot][h2 : h2 + 64, bl * LQ : (bl + 1) * LQ],
                                start=True, stop=True, tile_position=(h2, 32 * j),
                            )
                            me = meap.tile([LK, LQ], BF, tag="mea")
                            nc.scalar.activation(me[:], sa[:], EXP, scale=SCALE)
                            nc.vector.tensor_mul(me[:], me[:], ma_sb[b0 + bl][:])
                            mexa.append(me)
                        meb = mebp.tile([QUAD * LQ, LQ], BF, tag="meb")
                        nc.scalar.activation(meb[:], sb4[:], EXP, scale=SCALE)
                        nc.vector.tensor_mul(meb[:], meb[:], mb_sb[gq][:])
                        ctxp = pctx.tile([128, HC], F32, tag="ctx")
                        for j in range(QUAD):
                            gb = gq * QUAD + j
                            nc.tensor.matmul(
                                ctxp[32 * j : 32 * (j + 1), :],
                                mexa[j][:],
                                vha_sb[gb][:, h * HC : (h + 1) * HC],
                                start=True, stop=False, tile_position=(0, 32 * j),
                            )
                            nc.tensor.matmul(
                                ctxp[32 * j : 32 * (j + 1), :],
                                meb[32 * j : 32 * (j + 1), :],
                                vhb_sb[gq][32 * j : 32 * (j + 1), h * HC : (h + 1) * HC],
                                start=False, stop=True, tile_position=(32 * j, 32 * j),
                            )
                        r = mscp.tile([128, 1], F32, tag="r")
                        nc.vector.tensor_scalar_add(r[:], ctxp[:, DK : DK + 1], 1e-8)
                        nc.vector.reciprocal(r[:], r[:])
                        nc.vector.tensor_scalar_mul(
                            outq[:, h * DK : (h + 1) * DK], ctxp[:, 0:DK], r[:]
                        )
                    nc.sync.dma_start(outd[gq * 128 : (gq + 1) * 128, :], outq[:])

    nc.compile()
    return nc


def _get_nc():
    global _NC
    if _NC is None:
        _NC = _build()
    return _NC


def kernel(**inputs):
    global _LAST_RESULT
    Q_seq = np.asarray(inputs["Q_seq"], dtype=np.float32)
    K_seq = np.asarray(inputs["K_seq"], dtype=np.float32)
    V_seq = np.asarray(inputs["V_seq"], dtype=np.float32)
    tm = np.asarray(inputs["title_mask"], dtype=np.float32)
    bm = np.asarray(inputs["body_mask"], dtype=np.float32)
    Wq = np.asarray(inputs["Wq"], dtype=np.float32)
    Wk = np.asarray(inputs["Wk"], dtype=np.float32)
    Wv = np.asarray(inputs["Wv"], dtype=np.float32)
    bq = np.asarray(inputs["bq"], dtype=np.float32)
    bk = np.asarray(inputs["bk"], dtype=np.float32)
    bv = np.asarray(inputs["bv"], dtype=np.float32)

    bf = ml_dtypes.bfloat16
    # K_cat = concat(K_seq, Q_seq); V_cat = concat(V_seq, Q_seq). The V
    # projection of the shared Q_seq rows reuses xt's query columns, so
    # xvt only carries the V_seq part.
    Xk = np.concatenate([K_seq, Q_seq], axis=1)  # [B, L, D]

    wqt = np.ascontiguousarray(Wq.T).astype(bf)
    wkt = np.ascontiguousarray(Wk.T).astype(bf)
    wvt = np.ascontiguousarray(Wv.T).astype(bf)

    maska = np.ascontiguousarray((bm * tm[:, :, None]).transpose(0, 2, 1)).astype(bf)  # [B,128,32]
    maskb = (tm[:, :, None] * tm[:, None, :]).astype(bf)  # [B, 32(i), 32(q)]

    nc = _get_nc()
    in_maps = []
    for c in range(NCORES):
        sl = slice(c * NB, (c + 1) * NB)
        XT = np.ascontiguousarray(Xk[sl].reshape(NB * L, D).T).astype(bf)
        XVT = np.ascontiguousarray(V_seq[sl].reshape(NB * LK, D).T).astype(bf)
        XQT = np.ascontiguousarray(Q_seq[sl].reshape(NB * LQ, D).T).astype(bf)
        in_maps.append({
            "xt": XT,
            "xvt": XVT,
            "xqt": XQT,
            "wqt": wqt, "wkt": wkt, "wvt": wvt,
            "bq": np.ascontiguousarray(bq.reshape(8, 128, 1)),
            "bk": np.ascontiguousarray(bk.reshape(8, 128, 1)),
            "bvrow": np.ascontiguousarray(bv.reshape(1, D)).astype(bf),
            "maska": np.ascontiguousarray(maska[sl]),
            "maskb": np.ascontiguousarray(maskb[sl].reshape(NB // QUAD, QUAD * LQ, LQ)),
        })

    res = run_bass_kernel_spmd(nc, in_maps, core_ids=list(range(NCORES)))
    _LAST_RESULT = res
    out = np.concatenate(
        [res.results[c]["out"].reshape(NB, LQ, D) for c in range(NCORES)], axis=0
    )
    return np.ascontiguousarray(out.astype(np.float32))
